# revision 1
# baseline (speedup 1.0000x reference)
"""Trainium2 Bass kernel for KroneckerLinear: y = x @ kron(U, V).

Math: with x[t] reshaped to X_t [i1=128, i2=128] (i2 contiguous) and
y[t] reshaped to Y_t [j1=128, j2=128] (j2 contiguous):

    Y_t = U^T @ X_t @ V

Both stages map onto the PE array with the *token* tile as the stationary
operand (lhsT), so every tensor stays in its natural (contiguous) layout
and no transposes are needed anywhere:

    MM1: out = lhsT.T @ rhs with lhsT = X_t  [i1, i2], rhs = U [i1, j1]
         -> P^T [i2, j1]   (P = U^T X_t)
    MM2: lhsT = P^T [i2, j1], rhs = V [i2, j2]
         -> Y_t [j1, j2]

Sharding: data-parallel over the token dim, 256 tokens per core x 8 cores.
"""

import sys

if "/opt/trn_rl_repo" not in sys.path:
    sys.path.insert(0, "/opt/trn_rl_repo")

import numpy as np

import concourse.bacc as bacc
import concourse.bass as bass
import concourse.mybir as mybir
from concourse import tile
from concourse.bass_utils import run_bass_kernel_spmd

F32 = mybir.dt.float32
F32R = mybir.dt.float32r

N_CORES = 8
TOKENS = 2048
D = 16384  # 128 * 128
T_CORE = TOKENS // N_CORES  # 256


def build_nc(n_tokens=T_CORE, mode="fp32r", group=32, quad=4):
    """Build + compile the per-core program.

    mode:
      "fp32"  - exact fp32 matmuls (4 cycles/row on PE)
      "fp32r" - float32r matmuls with the moving operand padded to 256
                columns ([U|U], [V|V]) to hit the 1 cycle/row fast path
    """
    assert n_tokens % group == 0 and group % quad == 0
    rwide = 256 if mode == "fp32r" else 128

    nc = bacc.Bacc("TRN2", target_bir_lowering=False, debug=False)
    x = nc.dram_tensor("x", [n_tokens, D], F32, kind="ExternalInput")
    u = nc.dram_tensor("u", [128, rwide], F32, kind="ExternalInput")
    v = nc.dram_tensor("v", [128, rwide], F32, kind="ExternalInput")
    y = nc.dram_tensor("y", [n_tokens, D], F32, kind="ExternalOutput")

    def mmcast(ap):
        return ap.bitcast(F32R) if mode == "fp32r" else ap

    with tile.TileContext(nc) as tc:
        with (
            tc.tile_pool(name="const", bufs=1) as cpool,
            tc.tile_pool(name="xin", bufs=2) as xpool,
            tc.tile_pool(name="yout", bufs=2) as ypool,
            tc.tile_pool(name="pmid", bufs=4) as ppool,
            tc.tile_pool(name="ps", bufs=2, space="PSUM") as pspool,
        ):
            u_sb = cpool.tile([128, rwide], F32)
            v_sb = cpool.tile([128, rwide], F32)
            nc.sync.dma_start(u_sb[:], u[:])
            nc.sync.dma_start(v_sb[:], v[:])

            for g in range(n_tokens // group):
                xt = xpool.tile([128, group, 128], F32)
                nc.sync.dma_start(
                    xt[:],
                    x[g * group : (g + 1) * group].rearrange(
                        "t (i1 i2) -> i1 t i2", i1=128
                    ),
                )
                yt = ypool.tile([128, group, 128], F32)
                for q in range(group // quad):
                    pa = pspool.tile([128, quad, rwide], F32)
                    for j in range(quad):
                        nc.tensor.matmul(
                            pa[:, j, :],
                            lhsT=mmcast(xt[:, q * quad + j, :]),
                            rhs=mmcast(u_sb[:]),
                            start=True,
                            stop=True,
                        )
                    psb = ppool.tile([128, quad, 128], F32)
                    nc.vector.tensor_copy(psb[:], pa[:, :, 0:128])
                    pb = pspool.tile([128, quad, rwide], F32)
                    for j in range(quad):
                        nc.tensor.matmul(
                            pb[:, j, :],
                            lhsT=mmcast(psb[:, j, :]),
                            rhs=mmcast(v_sb[:]),
                            start=True,
                            stop=True,
                        )
                    nc.vector.tensor_copy(
                        yt[:, q * quad : (q + 1) * quad, :], pb[:, :, 0:128]
                    )
                nc.scalar.dma_start(
                    y[g * group : (g + 1) * group].rearrange(
                        "t (j1 j2) -> j1 t j2", j1=128
                    ),
                    yt[:],
                )
    nc.compile()
    return nc


_NC_CACHE = {}


def _get_nc(n_tokens, mode, group, quad):
    key = (n_tokens, mode, group, quad)
    if key not in _NC_CACHE:
        _NC_CACHE[key] = build_nc(n_tokens, mode, group, quad)
    return _NC_CACHE[key]


def _prep_inputs(x, U, V, mode):
    x = np.ascontiguousarray(np.asarray(x), dtype=np.float32)
    U = np.ascontiguousarray(np.asarray(U), dtype=np.float32)
    V = np.ascontiguousarray(np.asarray(V), dtype=np.float32)
    if mode == "fp32r":
        U = np.concatenate([U, U], axis=1)
        V = np.concatenate([V, V], axis=1)
    return x, U, V


def run(x, U, V, mode="fp32r", group=32, quad=4, trace=False, **spmd_kwargs):
    """Shard over 8 cores, run, gather. Returns (y_full, BassKernelResults)."""
    x, U, V = _prep_inputs(x, U, V, mode)
    t_core = x.shape[0] // N_CORES
    nc = _get_nc(t_core, mode, group, quad)
    in_maps = [
        {"x": x[i * t_core : (i + 1) * t_core], "u": U, "v": V}
        for i in range(N_CORES)
    ]
    res = run_bass_kernel_spmd(
        nc, in_maps, list(range(N_CORES)), trace=trace, **spmd_kwargs
    )
    out = np.concatenate([res.results[i]["y"] for i in range(N_CORES)], axis=0)
    return out, res


def kernel(x, U, V):
    out, _ = run(x, U, V)
    return out


# revision 2
# speedup vs baseline: 1.0319x; 1.0319x over previous
"""Trainium2 Bass kernel for KroneckerLinear: y = x @ kron(U, V).

Math: with x[t] reshaped to X_t [i1=128, i2=128] (i2 contiguous) and
y[t] reshaped to Y_t [j1=128, j2=128] (j2 contiguous):

    Y_t = U^T @ X_t @ V

Both stages map onto the PE array with the *token* tile as the stationary
operand (lhsT), so every tensor stays in its natural (contiguous) layout
and no transposes are needed anywhere:

    MM1: out = lhsT.T @ rhs with lhsT = X_t  [i1, i2], rhs = U [i1, j1]
         -> P^T [i2, j1]   (P = U^T X_t)
    MM2: lhsT = P^T [i2, j1], rhs = V [i2, j2]
         -> Y_t [j1, j2]

Sharding: data-parallel over the token dim, 256 tokens per core x 8 cores.
"""

import sys

if "/opt/trn_rl_repo" not in sys.path:
    sys.path.insert(0, "/opt/trn_rl_repo")

import numpy as np

import concourse.bacc as bacc
import concourse.bass as bass
import concourse.mybir as mybir
from concourse import tile
from concourse.bass_utils import run_bass_kernel_spmd

F32 = mybir.dt.float32
F32R = mybir.dt.float32r

N_CORES = 8
TOKENS = 2048
D = 16384  # 128 * 128
T_CORE = TOKENS // N_CORES  # 256


def build_nc(n_tokens=T_CORE, mode="fp32r", group=32, quad=4):
    """Build + compile the per-core program.

    mode:
      "fp32"  - exact fp32 matmuls (4 cycles/row on PE)
      "fp32r" - float32r matmuls with the moving operand padded to 256
                columns ([U|U], [V|V]) to hit the 1 cycle/row fast path.
                fp32r operands must be *produced* rounded: x tiles are
                cast during the (SWDGE) load, P tiles by the DVE copy.
    """
    assert n_tokens % group == 0 and group % quad == 0
    r = mode == "fp32r"
    rwide = 256 if r else 128
    mmdt = F32R if r else F32

    nc = bacc.Bacc("TRN2", target_bir_lowering=False, debug=False)
    x = nc.dram_tensor("x", [n_tokens, D], F32, kind="ExternalInput")
    u = nc.dram_tensor("u", [128, rwide], F32, kind="ExternalInput")
    v = nc.dram_tensor("v", [128, rwide], F32, kind="ExternalInput")
    y = nc.dram_tensor("y", [n_tokens, D], F32, kind="ExternalOutput")

    with tile.TileContext(nc) as tc:
        with (
            tc.tile_pool(name="const", bufs=1) as cpool,
            tc.tile_pool(name="xin", bufs=2) as xpool,
            tc.tile_pool(name="yout", bufs=2) as ypool,
            tc.tile_pool(name="pmid", bufs=4) as ppool,
            tc.tile_pool(name="ps", bufs=2, space="PSUM") as pspool,
        ):
            u_sb = cpool.tile([128, rwide], mmdt)
            v_sb = cpool.tile([128, rwide], mmdt)
            ld_const = nc.gpsimd.dma_start if r else nc.sync.dma_start
            ld_const(u_sb[:], u[:])
            ld_const(v_sb[:], v[:])

            for g in range(n_tokens // group):
                xt = xpool.tile([128, group, 128], mmdt)
                ld_x = nc.gpsimd.dma_start if r else nc.sync.dma_start
                ld_x(
                    xt[:],
                    x[g * group : (g + 1) * group].rearrange(
                        "t (i1 i2) -> i1 t i2", i1=128
                    ),
                )
                yt = ypool.tile([128, group, 128], F32)
                for q in range(group // quad):
                    pa = pspool.tile([128, quad, rwide], F32)
                    for j in range(quad):
                        nc.tensor.matmul(
                            pa[:, j, :],
                            lhsT=xt[:, q * quad + j, :],
                            rhs=u_sb[:],
                            start=True,
                            stop=True,
                        )
                    psb = ppool.tile([128, quad, 128], mmdt)
                    nc.vector.tensor_copy(psb[:], pa[:, :, 0:128])
                    pb = pspool.tile([128, quad, rwide], F32)
                    for j in range(quad):
                        nc.tensor.matmul(
                            pb[:, j, :],
                            lhsT=psb[:, j, :],
                            rhs=v_sb[:],
                            start=True,
                            stop=True,
                        )
                    nc.vector.tensor_copy(
                        yt[:, q * quad : (q + 1) * quad, :], pb[:, :, 0:128]
                    )
                nc.scalar.dma_start(
                    y[g * group : (g + 1) * group].rearrange(
                        "t (j1 j2) -> j1 t j2", j1=128
                    ),
                    yt[:],
                )
    nc.compile()
    return nc


_NC_CACHE = {}


def _get_nc(n_tokens, mode, group, quad):
    key = (n_tokens, mode, group, quad)
    if key not in _NC_CACHE:
        _NC_CACHE[key] = build_nc(n_tokens, mode, group, quad)
    return _NC_CACHE[key]


def _prep_inputs(x, U, V, mode):
    x = np.ascontiguousarray(np.asarray(x), dtype=np.float32)
    U = np.ascontiguousarray(np.asarray(U), dtype=np.float32)
    V = np.ascontiguousarray(np.asarray(V), dtype=np.float32)
    if mode == "fp32r":
        U = np.concatenate([U, U], axis=1)
        V = np.concatenate([V, V], axis=1)
    return x, U, V


def run(x, U, V, mode="fp32r", group=32, quad=4, trace=False, **spmd_kwargs):
    """Shard over 8 cores, run, gather. Returns (y_full, BassKernelResults)."""
    x, U, V = _prep_inputs(x, U, V, mode)
    t_core = x.shape[0] // N_CORES
    nc = _get_nc(t_core, mode, group, quad)
    in_maps = [
        {"x": x[i * t_core : (i + 1) * t_core], "u": U, "v": V}
        for i in range(N_CORES)
    ]
    res = run_bass_kernel_spmd(
        nc, in_maps, list(range(N_CORES)), trace=trace, **spmd_kwargs
    )
    out = np.concatenate([res.results[i]["y"] for i in range(N_CORES)], axis=0)
    return out, res


def kernel(x, U, V):
    out, _ = run(x, U, V)
    return out
